# revision 9
# baseline (speedup 1.0000x reference)
"""Trainium2 Bass kernel for nn_ExhustiveContrastiveLoss.

Reference computation (N=8192, D=512, fp32):
    xd = normalize(embed_data); xl = normalize(embed_label)
    f2f = xd @ xd.T with diagonal removed; e2p = xd @ xl.T (full)
    per-strip row max subtracted before exp (the two strips use DIFFERENT
    maxes inside the same num/den sums, so the maxes are semantically
    load-bearing, not just numerics)
    num = sum(pos * e2p_logits) + sum(pos_nd * f2f_logits)
    den = sum(e2p_logits) + sum(f2f_logits)
    loss = -mean(log(num / den))

Sharding: 4x2 grid over 8 cores. Core k: row shard r = k % 4 (2048 rows of
xd), col shard c = k // 4 (4096 columns of the similarity matrices).
Each core computes, for its [2048, 4096] block of both strips and each row:
    C' = sum_j exp((S - 1)/T)        (shift-1 denominator partial)
    A' = sum_j pos_ij exp((S - 1)/T) (shift-1 numerator partial)
    M  = max_j S                     (shard row max)
The host combines shards exactly as the reference would:
    num_s = exp((1 - Ms)/T) * (A'_c0 + A'_c1),  Ms = max over shards
(the fixed shift of 1.0 >= max cos keeps exp in [3.8e-13, 1] — fp32 safe).

The f2f diagonal is removed by accumulating -1e9 into the diagonal PSUM
cells via one extra identity-weight matmul whose rhs is a per-core host
input (all zeros on cores whose block does not contain the diagonal), so
the single SPMD program stays core-uniform.

Device work: all O(N^2) FLOPs (matmuls at float32r rate, exp, masked sums,
maxes). Host work is O(N*D) input prep (normalize + transpose + label
encode) and O(N) final combine.
"""

import os

os.environ.setdefault("MYCRO_LOCAL_CACHE", "1")

import numpy as np

import concourse.bass as bass
import concourse.bacc as bacc
import concourse.tile as tile
from concourse import mybir
from concourse.bass_utils import run_bass_kernel_spmd

# Problem constants (hardcoded per harness contract).
N, D = 8192, 512
NCORES = 8
RGRID, CGRID = 4, 2          # 4 row shards x 2 col shards
R = N // RGRID               # 2048 rows per core
C = N // CGRID               # 4096 cols per core
NIT = R // 128               # 16 row tiles
CHUNK = 1024                 # col chunk processed per inner step
NCTP = C // CHUNK            # 4 col chunks
TEMP = 0.07
EPS = 1e-8
SHIFT = 1.0                  # fixed exp shift; cos sim <= 1
NEG = -1.0e9

F32 = mybir.dt.float32
F32R = mybir.dt.float32r
BF16 = mybir.dt.bfloat16
AX = mybir.AxisListType
OP = mybir.AluOpType
AF = mybir.ActivationFunctionType


def _label_encode(lab):
    """Map class ids 0..999 to distinct bf16-exact floats."""
    lab = np.asarray(lab).astype(np.int64)
    return ((128 + (lab % 128)) * (2.0 ** (lab // 128))).astype(np.float32)


def build_nc():
    nc = bacc.Bacc(
        "TRN2",
        target_bir_lowering=False,
        debug=False,
        num_devices=NCORES,
    )

    lhs = nc.declare_dram_parameter("lhs", [D, R], F32R, isOutput=False)
    rhsD = nc.declare_dram_parameter("rhsD", [D, C], F32R, isOutput=False)
    rhsL = nc.declare_dram_parameter("rhsL", [D, C], F32R, isOutput=False)
    labs = nc.declare_dram_parameter("labs", [128, NIT], F32, isOutput=False)
    labc = nc.declare_dram_parameter("labc", [128, C], BF16, isOutput=False)
    mA = nc.declare_dram_parameter("mA", [128, 1152], F32R, isOutput=False)
    mB = nc.declare_dram_parameter("mB", [128, 1152], F32R, isOutput=False)
    eyeK = nc.declare_dram_parameter("eyeK", [128, 128], F32R, isOutput=False)

    outs = {
        name: nc.declare_dram_parameter(name, [128, NIT], F32, isOutput=True)
        for name in ("nf", "df", "mf", "ne", "de", "me")
    }

    with tile.TileContext(nc) as tc:
        with (
            tc.tile_pool(name="const", bufs=1) as const,
            tc.tile_pool(name="rhsp", bufs=2) as rhsp,
            tc.tile_pool(name="psum", bufs=3, space="PSUM") as psum,
            tc.tile_pool(name="lp", bufs=3) as lp,
            tc.tile_pool(name="mlp", bufs=2) as mlp,
            tc.tile_pool(name="statp", bufs=1) as statp,
            tc.tile_pool(name="outp", bufs=1) as outp,
        ):
            dma = nc.default_dma_engine

            lhs_sb = []
            for kc in range(4):
                t = const.tile([128, R], F32R, tag=f"lhs{kc}")
                dma.dma_start(out=t, in_=lhs[kc * 128:(kc + 1) * 128, :])
                lhs_sb.append(t)
            labs_sb = const.tile([128, NIT], F32, tag="labs")
            dma.dma_start(out=labs_sb, in_=labs[:, :])
            labc_sb = const.tile([128, C], BF16, tag="labc")
            dma.dma_start(out=labc_sb, in_=labc[:, :])
            mA_sb = const.tile([128, 1152], F32R, tag="mA")
            dma.dma_start(out=mA_sb, in_=mA[:, :])
            mB_sb = const.tile([128, 1152], F32R, tag="mB")
            dma.dma_start(out=mB_sb, in_=mB[:, :])
            eye_sb = const.tile([128, 128], F32R, tag="eyeK")
            dma.dma_start(out=eye_sb, in_=eyeK[:, :])
            bias_sb = const.tile([128, 1], F32, tag="expbias")
            nc.vector.memset(bias_sb, -SHIFT / TEMP)

            # per-(it, ctp) stat slots, reduced to per-it at the end
            stats = {}
            for sname in ("f", "e"):
                for kind in ("n", "d", "m"):
                    t = statp.tile([128, NIT * NCTP], F32, tag=f"st_{kind}{sname}")
                    stats[kind + sname] = t

            for ctp in range(NCTP):
                rhs_t = {}
                for mname, dram in (("f", rhsD), ("e", rhsL)):
                    tiles = []
                    for kc in range(4):
                        t = rhsp.tile([128, CHUNK], F32R, tag=f"rhs_{mname}{kc}")
                        dma.dma_start(
                            out=t,
                            in_=dram[kc * 128:(kc + 1) * 128,
                                     ctp * CHUNK:(ctp + 1) * CHUNK],
                        )
                        tiles.append(t)
                    rhs_t[mname] = tiles

                for it in range(NIT):
                    slot = slice(it * NCTP + ctp, it * NCTP + ctp + 1)

                    for sname in ("f", "e"):
                        ps = psum.tile([128, CHUNK], F32, tag="ps")
                        for nt in range(2):
                            ct = ctp * 2 + nt  # global 512-col tile index 0..7
                            # Diagonal mask membership for the f2f strip:
                            # variant A targets ct = it//4 (row shard even),
                            # variant B targets ct = 4 + it//4 (row shard odd).
                            mask_sb = None
                            if sname == "f":
                                if ct == it // 4:
                                    mask_sb = mA_sb
                                elif ct == 4 + it // 4:
                                    mask_sb = mB_sb
                            reg = ps[:, nt * 512:(nt + 1) * 512]
                            for kc in range(4):
                                nc.tensor.matmul(
                                    reg,
                                    lhsT=lhs_sb[kc][:, it * 128:(it + 1) * 128],
                                    rhs=rhs_t[sname][kc][:, nt * 512:(nt + 1) * 512],
                                    start=(kc == 0),
                                    stop=(kc == 3 and mask_sb is None),
                                )
                            if mask_sb is not None:
                                start_col = 512 - 128 * (it % 4)
                                nc.tensor.matmul(
                                    reg,
                                    lhsT=eye_sb,
                                    rhs=mask_sb[:, start_col:start_col + 512],
                                    start=False,
                                    stop=True,
                                )

                        l_t = lp.tile([128, CHUNK], BF16, tag="l")
                        nc.scalar.activation(
                            out=l_t,
                            in_=ps,
                            func=AF.Exp,
                            bias=bias_sb,
                            scale=1.0 / TEMP,
                            accum_out=stats["d" + sname][:, slot],
                        )
                        nc.vector.tensor_reduce(
                            out=stats["m" + sname][:, slot],
                            in_=ps,
                            axis=AX.X,
                            op=OP.max,
                        )
                        # num partial: ((labc == labs_it) * l) summed, one op
                        ml_t = mlp.tile([128, CHUNK], BF16, tag="ml")
                        nc.vector.scalar_tensor_tensor(
                            out=ml_t,
                            in0=labc_sb[:, ctp * CHUNK:(ctp + 1) * CHUNK],
                            scalar=labs_sb[:, it:it + 1],
                            in1=l_t,
                            op0=OP.is_equal,
                            op1=OP.mult,
                            accum_out=stats["n" + sname][:, slot],
                        )

            # reduce the per-ctp slots and ship out
            for sname in ("f", "e"):
                for kind, op in (("n", OP.add), ("d", OP.add), ("m", OP.max)):
                    o = outp.tile([128, NIT], F32, tag=f"o_{kind}{sname}")
                    nc.vector.tensor_reduce(
                        out=o,
                        in_=stats[kind + sname].rearrange(
                            "p (a b) -> p a b", b=NCTP
                        ),
                        axis=AX.X,
                        op=op,
                    )
                    dma.dma_start(out=outs[kind + sname][:, :], in_=o)

    nc.finalize()
    return nc


_NC_CACHE = None


def _get_nc():
    global _NC_CACHE
    if _NC_CACHE is None:
        _NC_CACHE = build_nc()
    return _NC_CACHE


def _prep_inputs(embed_data, embed_label, label):
    xd = np.asarray(embed_data, dtype=np.float32)
    xl = np.asarray(embed_label, dtype=np.float32)
    lab = np.asarray(label)

    def norm(x):
        n = np.sqrt(np.sum(x.astype(np.float64) ** 2, axis=1, keepdims=True))
        n = np.maximum(n, EPS)
        return (x / n).astype(np.float32)

    xdT = np.ascontiguousarray(norm(xd).T)  # [D, N]
    xlT = np.ascontiguousarray(norm(xl).T)
    labf = _label_encode(lab)               # [N] f32, bf16-exact values

    import ml_dtypes

    eyeK = np.eye(128, dtype=np.float32)

    in_maps = []
    for k in range(NCORES):
        r, c = k % RGRID, k // RGRID
        rows = slice(R * r, R * (r + 1))
        cols = slice(C * c, C * (c + 1))

        labs = labf[rows].reshape(NIT, 128).T.copy()      # [128, NIT]
        labc = np.broadcast_to(
            labf[cols].astype(ml_dtypes.bfloat16), (128, C)
        ).copy()

        mAb = np.zeros((128, 1152), dtype=np.float32)
        mBb = np.zeros((128, 1152), dtype=np.float32)
        if r // 2 == c:
            tgt = mAb if r % 2 == 0 else mBb
            tgt[np.arange(128), 512 + np.arange(128)] = NEG

        in_maps.append({
            "lhs": np.ascontiguousarray(xdT[:, rows]),
            "rhsD": np.ascontiguousarray(xdT[:, cols]),
            "rhsL": np.ascontiguousarray(xlT[:, cols]),
            "labs": np.ascontiguousarray(labs),
            "labc": labc,
            "mA": mAb,
            "mB": mBb,
            "eyeK": eyeK,
        })
    return in_maps


def _combine(results):
    """Host combine of per-core shard stats -> scalar loss (fp64)."""
    # stats[name][r][c] = [128, NIT]
    def get(name):
        out = np.empty((RGRID, CGRID, 128, NIT), dtype=np.float64)
        for k in range(NCORES):
            r, c = k % RGRID, k // RGRID
            out[r, c] = results[k][name].astype(np.float64)
        return out

    nf, df, mf = get("nf"), get("df"), get("mf")
    ne, de, me = get("ne"), get("de"), get("me")

    # row g = 2048 r + 128 it + p  <->  [r, c, p, it]
    Mf = np.max(mf, axis=1)            # [RGRID, 128, NIT]
    Me = np.max(me, axis=1)
    Af = np.sum(nf, axis=1)
    Cf = np.sum(df, axis=1)
    Ae = np.sum(ne, axis=1)
    Ce = np.sum(de, axis=1)

    wf = np.exp((SHIFT - Mf) / TEMP)
    we = np.exp((SHIFT - Me) / TEMP)
    num = we * Ae + wf * Af
    den = we * Ce + wf * Cf
    row_loss = np.log(den) - np.log(num)
    return np.float32(np.mean(row_loss))


def kernel(embed_data, embed_label, label):
    nc = _get_nc()
    in_maps = _prep_inputs(embed_data, embed_label, label)
    res = run_bass_kernel_spmd(nc, in_maps, list(range(NCORES)))
    return _combine(res.results)


if __name__ == "__main__":
    rng = np.random.default_rng(0)
    ed = rng.standard_normal((N, D), dtype=np.float32)
    el = rng.standard_normal((N, D), dtype=np.float32)
    lb = rng.integers(0, 1000, N)
    print(kernel(ed, el, lb))


# revision 13
# speedup vs baseline: 1.1342x; 1.1342x over previous
"""Trainium2 Bass kernel for nn_ExhustiveContrastiveLoss.

Reference computation (N=8192, D=512, fp32):
    xd = normalize(embed_data); xl = normalize(embed_label)
    f2f = xd @ xd.T with diagonal removed; e2p = xd @ xl.T (full)
    per-strip row max subtracted before exp (the two strips use DIFFERENT
    maxes inside the same num/den sums, so the maxes are semantically
    load-bearing, not just numerics)
    num = sum(pos * e2p_logits) + sum(pos_nd * f2f_logits)
    den = sum(e2p_logits) + sum(f2f_logits)
    loss = -mean(log(num / den))

Sharding: 4x2 grid over 8 cores. Core k: row shard r = k % 4 (2048 rows of
xd), col shard c = k // 4 (4096 columns of the similarity matrices).
Each core computes, for its [2048, 4096] block of both strips and each row:
    C' = sum_j exp((S - 1)/T)        (shift-1 denominator partial)
    A' = sum_j pos_ij exp((S - 1)/T) (shift-1 numerator partial)
    M  = max_j S                     (shard row max)
The host combines shards exactly as the reference would:
    num_s = exp((1 - Ms)/T) * (A'_c0 + A'_c1),  Ms = max over shards
(the fixed shift of 1.0 >= max cos keeps exp in [3.8e-13, 1] — fp32 safe).

The f2f diagonal is removed by accumulating -1e9 into the diagonal PSUM
cells via one extra identity-weight matmul whose rhs is a per-core host
input (all zeros on cores whose block does not contain the diagonal), so
the single SPMD program stays core-uniform.

Device work: all O(N^2) FLOPs (matmuls at float32r rate, exp, masked sums,
maxes). Host work is O(N*D) input prep (normalize + transpose + label
encode) and O(N) final combine.
"""

import os

os.environ.setdefault("MYCRO_LOCAL_CACHE", "1")

import numpy as np

import concourse.bass as bass
import concourse.bacc as bacc
import concourse.tile as tile
from concourse import mybir
from concourse.bass_utils import run_bass_kernel_spmd

# Problem constants (hardcoded per harness contract).
N, D = 8192, 512
NCORES = 8
RGRID, CGRID = 4, 2          # 4 row shards x 2 col shards
R = N // RGRID               # 2048 rows per core
C = N // CGRID               # 4096 cols per core
NIT = R // 128               # 16 row tiles
CHUNK = 1024                 # col chunk processed per inner step
NCTP = C // CHUNK            # 4 col chunks
TEMP = 0.07
EPS = 1e-8
SHIFT = 1.0                  # fixed exp shift; cos sim <= 1
NEG = -1.0e9
GPSIMD_STT = False           # offload e2p numerator stt to GPSIMD

F32 = mybir.dt.float32
F32R = mybir.dt.float32r
BF16 = mybir.dt.bfloat16
AX = mybir.AxisListType
OP = mybir.AluOpType
AF = mybir.ActivationFunctionType


def _label_encode(lab):
    """Map class ids 0..999 to distinct bf16-exact floats."""
    lab = np.asarray(lab).astype(np.int64)
    return ((128 + (lab % 128)) * (2.0 ** (lab // 128))).astype(np.float32)


def build_nc():
    nc = bacc.Bacc(
        "TRN2",
        target_bir_lowering=False,
        debug=False,
        num_devices=NCORES,
    )

    lhs = nc.declare_dram_parameter("lhs", [D, R], F32R, isOutput=False)
    rhsD = nc.declare_dram_parameter("rhsD", [D, C], F32R, isOutput=False)
    rhsL = nc.declare_dram_parameter("rhsL", [D, C], F32R, isOutput=False)
    labs = nc.declare_dram_parameter("labs", [128, NIT], F32, isOutput=False)
    labc = nc.declare_dram_parameter("labc", [128, C], BF16, isOutput=False)
    mA = nc.declare_dram_parameter("mA", [128, 1152], F32R, isOutput=False)
    mB = nc.declare_dram_parameter("mB", [128, 1152], F32R, isOutput=False)
    eyeK = nc.declare_dram_parameter("eyeK", [128, 128], F32R, isOutput=False)

    outs = {
        name: nc.declare_dram_parameter(name, [128, NIT], F32, isOutput=True)
        for name in ("nf", "df", "mf", "ne", "de", "me")
    }

    with tile.TileContext(nc) as tc:
        with (
            tc.tile_pool(name="const", bufs=1) as const,
            tc.tile_pool(name="rhsp", bufs=2) as rhsp,
            tc.tile_pool(name="psum", bufs=4, space="PSUM") as psum,
            tc.tile_pool(name="lp", bufs=3) as lp,
            tc.tile_pool(name="mlp", bufs=2) as mlp,
            tc.tile_pool(name="mtp", bufs=2) as mtp,
            tc.tile_pool(name="statp", bufs=1) as statp,
            tc.tile_pool(name="outp", bufs=1) as outp,
        ):
            dma = nc.default_dma_engine

            lhs_sb = []
            for kc in range(4):
                t = const.tile([128, R], F32R, tag=f"lhs{kc}")
                dma.dma_start(out=t, in_=lhs[kc * 128:(kc + 1) * 128, :])
                lhs_sb.append(t)
            labs_sb = const.tile([128, NIT], F32, tag="labs")
            dma.dma_start(out=labs_sb, in_=labs[:, :])
            labc_sb = const.tile([128, C], BF16, tag="labc")
            dma.dma_start(out=labc_sb, in_=labc[:, :])
            mA_sb = const.tile([128, 1152], F32R, tag="mA")
            dma.dma_start(out=mA_sb, in_=mA[:, :])
            mB_sb = const.tile([128, 1152], F32R, tag="mB")
            dma.dma_start(out=mB_sb, in_=mB[:, :])
            eye_sb = const.tile([128, 128], F32R, tag="eyeK")
            dma.dma_start(out=eye_sb, in_=eyeK[:, :])
            bias_sb = const.tile([128, 1], F32, tag="expbias")
            nc.vector.memset(bias_sb, -SHIFT / TEMP)

            # per-(it, ctp) stat slots, reduced to per-it at the end
            stats = {}
            for sname in ("f", "e"):
                for kind in ("n", "d", "m"):
                    t = statp.tile([128, NIT * NCTP], F32, tag=f"st_{kind}{sname}")
                    stats[kind + sname] = t

            for ctp in range(NCTP):
                rhs_t = {}
                for mname, dram in (("f", rhsD), ("e", rhsL)):
                    tiles = []
                    for kc in range(4):
                        t = rhsp.tile([128, CHUNK], F32R, tag=f"rhs_{mname}{kc}")
                        dma.dma_start(
                            out=t,
                            in_=dram[kc * 128:(kc + 1) * 128,
                                     ctp * CHUNK:(ctp + 1) * CHUNK],
                        )
                        tiles.append(t)
                    rhs_t[mname] = tiles

                for it in range(NIT):
                    slot = slice(it * NCTP + ctp, it * NCTP + ctp + 1)

                    for sname in ("f", "e"):
                        ps = psum.tile([128, CHUNK], F32, tag="ps")
                        for nt in range(2):
                            ct = ctp * 2 + nt  # global 512-col tile index 0..7
                            # Diagonal mask membership for the f2f strip:
                            # variant A targets ct = it//4 (row shard even),
                            # variant B targets ct = 4 + it//4 (row shard odd).
                            mask_sb = None
                            if sname == "f":
                                if ct == it // 4:
                                    mask_sb = mA_sb
                                elif ct == 4 + it // 4:
                                    mask_sb = mB_sb
                            reg = ps[:, nt * 512:(nt + 1) * 512]
                            for kc in range(4):
                                nc.tensor.matmul(
                                    reg,
                                    lhsT=lhs_sb[kc][:, it * 128:(it + 1) * 128],
                                    rhs=rhs_t[sname][kc][:, nt * 512:(nt + 1) * 512],
                                    start=(kc == 0),
                                    stop=(kc == 3 and mask_sb is None),
                                )
                            if mask_sb is not None:
                                start_col = 512 - 128 * (it % 4)
                                nc.tensor.matmul(
                                    reg,
                                    lhsT=eye_sb,
                                    rhs=mask_sb[:, start_col:start_col + 512],
                                    start=False,
                                    stop=True,
                                )

                        l_t = lp.tile([128, CHUNK], BF16, tag="l")
                        nc.scalar.activation(
                            out=l_t,
                            in_=ps,
                            func=AF.Exp,
                            bias=bias_sb,
                            scale=1.0 / TEMP,
                            accum_out=stats["d" + sname][:, slot],
                        )
                        # row max of l' (bf16, 2x TT tree); host recovers
                        # Ms = SHIFT + T*ln(max_l)
                        m1 = mtp.tile([128, 512], BF16, tag="m1")
                        nc.vector.tensor_tensor(
                            out=m1, in0=l_t[:, :512], in1=l_t[:, 512:],
                            op=OP.max,
                        )
                        m2 = mtp.tile([128, 256], BF16, tag="m2")
                        nc.vector.tensor_tensor(
                            out=m2, in0=m1[:, :256], in1=m1[:, 256:],
                            op=OP.max,
                        )
                        nc.vector.tensor_reduce(
                            out=stats["m" + sname][:, slot],
                            in_=m2,
                            axis=AX.X,
                            op=OP.max,
                        )
                        # num partial: ((labc == labs_it) * l) summed, one op
                        eng = nc.gpsimd if (GPSIMD_STT and sname == "e") else nc.vector
                        ml_t = mlp.tile([128, CHUNK], BF16, tag="ml")
                        eng.scalar_tensor_tensor(
                            out=ml_t,
                            in0=labc_sb[:, ctp * CHUNK:(ctp + 1) * CHUNK],
                            scalar=labs_sb[:, it:it + 1],
                            in1=l_t,
                            op0=OP.is_equal,
                            op1=OP.mult,
                            accum_out=stats["n" + sname][:, slot],
                        )

            # reduce the per-ctp slots and ship out
            for sname in ("f", "e"):
                for kind, op in (("n", OP.add), ("d", OP.add), ("m", OP.max)):
                    o = outp.tile([128, NIT], F32, tag=f"o_{kind}{sname}")
                    nc.vector.tensor_reduce(
                        out=o,
                        in_=stats[kind + sname].rearrange(
                            "p (a b) -> p a b", b=NCTP
                        ),
                        axis=AX.X,
                        op=op,
                    )
                    dma.dma_start(out=outs[kind + sname][:, :], in_=o)

    nc.finalize()
    return nc


_NC_CACHE = None


def _get_nc():
    global _NC_CACHE
    if _NC_CACHE is None:
        _NC_CACHE = build_nc()
    return _NC_CACHE


def _prep_inputs(embed_data, embed_label, label):
    xd = np.asarray(embed_data, dtype=np.float32)
    xl = np.asarray(embed_label, dtype=np.float32)
    lab = np.asarray(label)

    def norm(x):
        n = np.sqrt(np.sum(x.astype(np.float64) ** 2, axis=1, keepdims=True))
        n = np.maximum(n, EPS)
        return (x / n).astype(np.float32)

    xdT = np.ascontiguousarray(norm(xd).T)  # [D, N]
    xlT = np.ascontiguousarray(norm(xl).T)
    labf = _label_encode(lab)               # [N] f32, bf16-exact values

    import ml_dtypes

    eyeK = np.eye(128, dtype=np.float32)

    in_maps = []
    for k in range(NCORES):
        r, c = k % RGRID, k // RGRID
        rows = slice(R * r, R * (r + 1))
        cols = slice(C * c, C * (c + 1))

        labs = labf[rows].reshape(NIT, 128).T.copy()      # [128, NIT]
        labc = np.broadcast_to(
            labf[cols].astype(ml_dtypes.bfloat16), (128, C)
        ).copy()

        mAb = np.zeros((128, 1152), dtype=np.float32)
        mBb = np.zeros((128, 1152), dtype=np.float32)
        if r // 2 == c:
            tgt = mAb if r % 2 == 0 else mBb
            tgt[np.arange(128), 512 + np.arange(128)] = NEG

        in_maps.append({
            "lhs": np.ascontiguousarray(xdT[:, rows]),
            "rhsD": np.ascontiguousarray(xdT[:, cols]),
            "rhsL": np.ascontiguousarray(xlT[:, cols]),
            "labs": np.ascontiguousarray(labs),
            "labc": labc,
            "mA": mAb,
            "mB": mBb,
            "eyeK": eyeK,
        })
    return in_maps


def _combine(results):
    """Host combine of per-core shard stats -> scalar loss (fp64)."""
    # stats[name][r][c] = [128, NIT]
    def get(name):
        out = np.empty((RGRID, CGRID, 128, NIT), dtype=np.float64)
        for k in range(NCORES):
            r, c = k % RGRID, k // RGRID
            out[r, c] = results[k][name].astype(np.float64)
        return out

    nf, df, mf = get("nf"), get("df"), get("mf")
    ne, de, me = get("ne"), get("de"), get("me")

    # row g = 2048 r + 128 it + p  <->  [r, c, p, it]
    # mf/me hold max_j l' = exp((Ms - SHIFT)/T); the reference weight
    # e^{(SHIFT - Ms)/T} is just its reciprocal.
    Mlf = np.max(mf, axis=1)           # [RGRID, 128, NIT]
    Mle = np.max(me, axis=1)
    Af = np.sum(nf, axis=1)
    Cf = np.sum(df, axis=1)
    Ae = np.sum(ne, axis=1)
    Ce = np.sum(de, axis=1)

    wf = 1.0 / Mlf
    we = 1.0 / Mle
    num = we * Ae + wf * Af
    den = we * Ce + wf * Cf
    row_loss = np.log(den) - np.log(num)
    return np.float32(np.mean(row_loss))


def kernel(embed_data, embed_label, label):
    nc = _get_nc()
    in_maps = _prep_inputs(embed_data, embed_label, label)
    res = run_bass_kernel_spmd(nc, in_maps, list(range(NCORES)))
    return _combine(res.results)


if __name__ == "__main__":
    rng = np.random.default_rng(0)
    ed = rng.standard_normal((N, D), dtype=np.float32)
    el = rng.standard_normal((N, D), dtype=np.float32)
    lb = rng.integers(0, 1000, N)
    print(kernel(ed, el, lb))


# revision 18
# speedup vs baseline: 1.1935x; 1.0523x over previous
"""Trainium2 Bass kernel for nn_ExhustiveContrastiveLoss.

Reference computation (N=8192, D=512, fp32):
    xd = normalize(embed_data); xl = normalize(embed_label)
    f2f = xd @ xd.T with diagonal removed; e2p = xd @ xl.T (full)
    per-strip row max subtracted before exp (the two strips use DIFFERENT
    maxes inside the same num/den sums, so the maxes are semantically
    load-bearing, not just numerics)
    num = sum(pos * e2p_logits) + sum(pos_nd * f2f_logits)
    den = sum(e2p_logits) + sum(f2f_logits)
    loss = -mean(log(num / den))

Sharding: 4x2 grid over 8 cores. Core k: row shard r = k % 4 (2048 rows of
xd), col shard c = k // 4 (4096 columns of the similarity matrices).
Each core computes, for its [2048, 4096] block of both strips and each row:
    C' = sum_j exp((S - 1)/T)        (shift-1 denominator partial)
    A' = sum_j pos_ij exp((S - 1)/T) (shift-1 numerator partial)
    M  = max_j S                     (shard row max)
The host combines shards exactly as the reference would:
    num_s = exp((1 - Ms)/T) * (A'_c0 + A'_c1),  Ms = max over shards
(the fixed shift of 1.0 >= max cos keeps exp in [3.8e-13, 1] — fp32 safe).

The f2f diagonal is removed by accumulating -1e9 into the diagonal PSUM
cells via one extra identity-weight matmul whose rhs is a per-core host
input (all zeros on cores whose block does not contain the diagonal), so
the single SPMD program stays core-uniform.

Device work: all O(N^2) FLOPs (matmuls at float32r rate, exp, masked sums,
maxes). Host work is O(N*D) input prep (normalize + transpose + label
encode) and O(N) final combine.
"""

import os

os.environ.setdefault("MYCRO_LOCAL_CACHE", "1")

import numpy as np

import concourse.bass as bass
import concourse.bacc as bacc
import concourse.tile as tile
from concourse import mybir
from concourse.bass_utils import run_bass_kernel_spmd

# Problem constants (hardcoded per harness contract).
N, D = 8192, 512
NCORES = 8
RGRID, CGRID = 4, 2          # 4 row shards x 2 col shards
R = N // RGRID               # 2048 rows per core
C = N // CGRID               # 4096 cols per core
NIT = R // 128               # 16 row tiles
CHUNK = 2048                 # col chunk processed per inner step
NCTP = C // CHUNK            # 4 col chunks
TEMP = 0.07
EPS = 1e-8
SHIFT = 1.0                  # fixed exp shift; cos sim <= 1
NEG = -1.0e9
GPSIMD_STT = False           # gpsimd lacks scalar_tensor_tensor codegen

F32 = mybir.dt.float32
F32R = mybir.dt.float32r
BF16 = mybir.dt.bfloat16
AX = mybir.AxisListType
OP = mybir.AluOpType
AF = mybir.ActivationFunctionType


def _label_encode(lab):
    """Map class ids 0..999 to distinct bf16-exact floats."""
    lab = np.asarray(lab).astype(np.int64)
    return ((128 + (lab % 128)) * (2.0 ** (lab // 128))).astype(np.float32)


def build_nc():
    nc = bacc.Bacc(
        "TRN2",
        target_bir_lowering=False,
        debug=False,
        num_devices=NCORES,
    )

    lhs = nc.declare_dram_parameter("lhs", [D, R], F32R, isOutput=False)
    rhsD = nc.declare_dram_parameter("rhsD", [D, C], F32R, isOutput=False)
    rhsL = nc.declare_dram_parameter("rhsL", [D, C], F32R, isOutput=False)
    labs = nc.declare_dram_parameter("labs", [128, NIT], F32, isOutput=False)
    labc = nc.declare_dram_parameter("labc", [128, C], BF16, isOutput=False)
    mA = nc.declare_dram_parameter("mA", [128, 1152], F32R, isOutput=False)
    mB = nc.declare_dram_parameter("mB", [128, 1152], F32R, isOutput=False)
    eyeK = nc.declare_dram_parameter("eyeK", [128, 128], F32R, isOutput=False)

    outs = {
        name: nc.declare_dram_parameter(name, [128, NIT], F32, isOutput=True)
        for name in ("nf", "df", "mf", "ne", "de", "me")
    }

    with tile.TileContext(nc) as tc:
        with (
            tc.tile_pool(name="const", bufs=1) as const,
            tc.tile_pool(name="rhsp", bufs=2) as rhsp,
            tc.tile_pool(name="psum", bufs=2, space="PSUM") as psum,
            tc.tile_pool(name="lp", bufs=2) as lp,
            tc.tile_pool(name="mlp", bufs=1) as mlp,
            tc.tile_pool(name="mtp", bufs=1) as mtp,
            tc.tile_pool(name="statp", bufs=1) as statp,
            tc.tile_pool(name="outp", bufs=1) as outp,
        ):
            dma = nc.default_dma_engine

            # lhs loaded in 512-col chunks so the first matmuls can start
            # after ~3 MiB of DMA instead of the full 12 MiB.
            lhs_sb = []
            for kc in range(4):
                t = const.tile([128, R], F32R, tag=f"lhs{kc}")
                dma.dma_start(
                    out=t[:, 0:512], in_=lhs[kc * 128:(kc + 1) * 128, 0:512]
                )
                lhs_sb.append(t)
            eye_sb = const.tile([128, 128], F32R, tag="eyeK")
            dma.dma_start(out=eye_sb, in_=eyeK[:, :])
            mA_sb = const.tile([128, 1152], F32R, tag="mA")
            dma.dma_start(out=mA_sb, in_=mA[:, :])
            mB_sb = const.tile([128, 1152], F32R, tag="mB")
            dma.dma_start(out=mB_sb, in_=mB[:, :])
            labs_sb = const.tile([128, NIT], F32, tag="labs")
            labc_sb = const.tile([128, C], BF16, tag="labc")
            bias_sb = const.tile([128, 1], F32, tag="expbias")
            nc.vector.memset(bias_sb, -SHIFT / TEMP)

            # per-(it, ctp) stat slots, reduced to per-it at the end
            stats = {}
            for sname in ("f", "e"):
                for kind in ("n", "d", "m"):
                    t = statp.tile([128, NIT * NCTP], F32, tag=f"st_{kind}{sname}")
                    stats[kind + sname] = t

            for ctp in range(NCTP):
                rhs_t = {}
                for mname, dram in (("f", rhsD), ("e", rhsL)):
                    tiles = []
                    for kc in range(4):
                        t = rhsp.tile([128, CHUNK], F32R, tag=f"rhs_{mname}{kc}")
                        dma.dma_start(
                            out=t,
                            in_=dram[kc * 128:(kc + 1) * 128,
                                     ctp * CHUNK:(ctp + 1) * CHUNK],
                        )
                        tiles.append(t)
                    rhs_t[mname] = tiles

                if ctp == 0:
                    # deferred loads: needed only a few tiles into the loop
                    dma.dma_start(out=labs_sb, in_=labs[:, :])
                    dma.dma_start(out=labc_sb, in_=labc[:, :])
                    for kc in range(4):
                        for part in range(1, 4):
                            sl = slice(part * 512, (part + 1) * 512)
                            dma.dma_start(
                                out=lhs_sb[kc][:, sl],
                                in_=lhs[kc * 128:(kc + 1) * 128, sl],
                            )

                for it in range(NIT):
                    slot = slice(it * NCTP + ctp, it * NCTP + ctp + 1)

                    for sname in ("f", "e"):
                        ps = psum.tile([128, CHUNK], F32, tag="ps")
                        for nt in range(CHUNK // 512):
                            ct = ctp * (CHUNK // 512) + nt  # global 512-col tile
                            # Diagonal mask membership for the f2f strip:
                            # variant A targets ct = it//4 (row shard even),
                            # variant B targets ct = 4 + it//4 (row shard odd).
                            mask_sb = None
                            if sname == "f":
                                if ct == it // 4:
                                    mask_sb = mA_sb
                                elif ct == 4 + it // 4:
                                    mask_sb = mB_sb
                            reg = ps[:, nt * 512:(nt + 1) * 512]
                            for kc in range(4):
                                nc.tensor.matmul(
                                    reg,
                                    lhsT=lhs_sb[kc][:, it * 128:(it + 1) * 128],
                                    rhs=rhs_t[sname][kc][:, nt * 512:(nt + 1) * 512],
                                    start=(kc == 0),
                                    stop=(kc == 3 and mask_sb is None),
                                )
                            if mask_sb is not None:
                                start_col = 512 - 128 * (it % 4)
                                nc.tensor.matmul(
                                    reg,
                                    lhsT=eye_sb,
                                    rhs=mask_sb[:, start_col:start_col + 512],
                                    start=False,
                                    stop=True,
                                )

                        l_t = lp.tile([128, CHUNK], BF16, tag="l")
                        nc.scalar.activation(
                            out=l_t,
                            in_=ps,
                            func=AF.Exp,
                            bias=bias_sb,
                            scale=1.0 / TEMP,
                            accum_out=stats["d" + sname][:, slot],
                        )
                        # row max of l' (bf16, 2x TT tree); host recovers
                        # Ms = SHIFT + T*ln(max_l)
                        m1 = mtp.tile([128, 1024], BF16, tag="m1")
                        nc.vector.tensor_tensor(
                            out=m1, in0=l_t[:, :1024], in1=l_t[:, 1024:],
                            op=OP.max,
                        )
                        m2 = mtp.tile([128, 512], BF16, tag="m2")
                        nc.vector.tensor_tensor(
                            out=m2, in0=m1[:, :512], in1=m1[:, 512:],
                            op=OP.max,
                        )
                        m3 = mtp.tile([128, 256], BF16, tag="m3")
                        nc.vector.tensor_tensor(
                            out=m3, in0=m2[:, :256], in1=m2[:, 256:],
                            op=OP.max,
                        )
                        nc.vector.tensor_reduce(
                            out=stats["m" + sname][:, slot],
                            in_=m3,
                            axis=AX.X,
                            op=OP.max,
                        )
                        # num partial: ((labc == labs_it) * l) summed, one op
                        eng = nc.gpsimd if (GPSIMD_STT and sname == "e") else nc.vector
                        ml_t = mlp.tile([128, CHUNK], BF16, tag="ml")
                        eng.scalar_tensor_tensor(
                            out=ml_t,
                            in0=labc_sb[:, ctp * CHUNK:(ctp + 1) * CHUNK],
                            scalar=labs_sb[:, it:it + 1],
                            in1=l_t,
                            op0=OP.is_equal,
                            op1=OP.mult,
                            accum_out=stats["n" + sname][:, slot],
                        )

            # reduce the per-ctp slots and ship out
            for sname in ("f", "e"):
                for kind, op in (("n", OP.add), ("d", OP.add), ("m", OP.max)):
                    o = outp.tile([128, NIT], F32, tag=f"o_{kind}{sname}")
                    nc.vector.tensor_reduce(
                        out=o,
                        in_=stats[kind + sname].rearrange(
                            "p (a b) -> p a b", b=NCTP
                        ),
                        axis=AX.X,
                        op=op,
                    )
                    dma.dma_start(out=outs[kind + sname][:, :], in_=o)

    nc.finalize()
    return nc


_NC_CACHE = None


def _get_nc():
    global _NC_CACHE
    if _NC_CACHE is None:
        _NC_CACHE = build_nc()
    return _NC_CACHE


def _prep_inputs(embed_data, embed_label, label):
    xd = np.asarray(embed_data, dtype=np.float32)
    xl = np.asarray(embed_label, dtype=np.float32)
    lab = np.asarray(label)

    def norm(x):
        n = np.sqrt(np.sum(x.astype(np.float64) ** 2, axis=1, keepdims=True))
        n = np.maximum(n, EPS)
        return (x / n).astype(np.float32)

    xdT = np.ascontiguousarray(norm(xd).T)  # [D, N]
    xlT = np.ascontiguousarray(norm(xl).T)
    labf = _label_encode(lab)               # [N] f32, bf16-exact values

    import ml_dtypes

    eyeK = np.eye(128, dtype=np.float32)

    in_maps = []
    for k in range(NCORES):
        r, c = k % RGRID, k // RGRID
        rows = slice(R * r, R * (r + 1))
        cols = slice(C * c, C * (c + 1))

        labs = labf[rows].reshape(NIT, 128).T.copy()      # [128, NIT]
        labc = np.broadcast_to(
            labf[cols].astype(ml_dtypes.bfloat16), (128, C)
        ).copy()

        mAb = np.zeros((128, 1152), dtype=np.float32)
        mBb = np.zeros((128, 1152), dtype=np.float32)
        if r // 2 == c:
            tgt = mAb if r % 2 == 0 else mBb
            tgt[np.arange(128), 512 + np.arange(128)] = NEG

        in_maps.append({
            "lhs": np.ascontiguousarray(xdT[:, rows]),
            "rhsD": np.ascontiguousarray(xdT[:, cols]),
            "rhsL": np.ascontiguousarray(xlT[:, cols]),
            "labs": np.ascontiguousarray(labs),
            "labc": labc,
            "mA": mAb,
            "mB": mBb,
            "eyeK": eyeK,
        })
    return in_maps


def _combine(results):
    """Host combine of per-core shard stats -> scalar loss (fp64)."""
    # stats[name][r][c] = [128, NIT]
    def get(name):
        out = np.empty((RGRID, CGRID, 128, NIT), dtype=np.float64)
        for k in range(NCORES):
            r, c = k % RGRID, k // RGRID
            out[r, c] = results[k][name].astype(np.float64)
        return out

    nf, df, mf = get("nf"), get("df"), get("mf")
    ne, de, me = get("ne"), get("de"), get("me")

    # row g = 2048 r + 128 it + p  <->  [r, c, p, it]
    # mf/me hold max_j l' = exp((Ms - SHIFT)/T); the reference weight
    # e^{(SHIFT - Ms)/T} is just its reciprocal.
    Mlf = np.max(mf, axis=1)           # [RGRID, 128, NIT]
    Mle = np.max(me, axis=1)
    Af = np.sum(nf, axis=1)
    Cf = np.sum(df, axis=1)
    Ae = np.sum(ne, axis=1)
    Ce = np.sum(de, axis=1)

    wf = 1.0 / Mlf
    we = 1.0 / Mle
    num = we * Ae + wf * Af
    den = we * Ce + wf * Cf
    row_loss = np.log(den) - np.log(num)
    return np.float32(np.mean(row_loss))


def kernel(embed_data, embed_label, label):
    nc = _get_nc()
    in_maps = _prep_inputs(embed_data, embed_label, label)
    res = run_bass_kernel_spmd(nc, in_maps, list(range(NCORES)))
    return _combine(res.results)


if __name__ == "__main__":
    rng = np.random.default_rng(0)
    ed = rng.standard_normal((N, D), dtype=np.float32)
    el = rng.standard_normal((N, D), dtype=np.float32)
    lb = rng.integers(0, 1000, N)
    print(kernel(ed, el, lb))


# revision 22
# speedup vs baseline: 1.2031x; 1.0081x over previous
"""Trainium2 Bass kernel for nn_ExhustiveContrastiveLoss.

Reference computation (N=8192, D=512, fp32):
    xd = normalize(embed_data); xl = normalize(embed_label)
    f2f = xd @ xd.T with diagonal removed; e2p = xd @ xl.T (full)
    per-strip row max subtracted before exp (the two strips use DIFFERENT
    maxes inside the same num/den sums, so the maxes are semantically
    load-bearing, not just numerics)
    num = sum(pos * e2p_logits) + sum(pos_nd * f2f_logits)
    den = sum(e2p_logits) + sum(f2f_logits)
    loss = -mean(log(num / den))

Sharding: 4x2 grid over 8 cores. Core k: row shard r = k % 4 (2048 rows of
xd), col shard c = k // 4 (4096 columns of the similarity matrices).
Each core computes, for its [2048, 4096] block of both strips and each row:
    C' = sum_j exp((S - 1)/T)        (shift-1 denominator partial)
    A' = sum_j pos_ij exp((S - 1)/T) (shift-1 numerator partial)
    M  = max_j S                     (shard row max)
The host combines shards exactly as the reference would:
    num_s = exp((1 - Ms)/T) * (A'_c0 + A'_c1),  Ms = max over shards
(the fixed shift of 1.0 >= max cos keeps exp in [3.8e-13, 1] — fp32 safe).

The f2f diagonal is removed by accumulating -1e9 into the diagonal PSUM
cells via one extra identity-weight matmul whose rhs is a per-core host
input (all zeros on cores whose block does not contain the diagonal), so
the single SPMD program stays core-uniform.

Device work: all O(N^2) FLOPs (matmuls at float32r rate, exp, masked sums,
maxes). Host work is O(N*D) input prep (normalize + transpose + label
encode) and O(N) final combine.
"""

import os

os.environ.setdefault("MYCRO_LOCAL_CACHE", "1")

import numpy as np

import concourse.bass as bass
import concourse.bacc as bacc
import concourse.tile as tile
from concourse import mybir
from concourse.bass_utils import run_bass_kernel_spmd

# Problem constants (hardcoded per harness contract).
N, D = 8192, 512
NCORES = 8
RGRID, CGRID = 4, 2          # 4 row shards x 2 col shards
R = N // RGRID               # 2048 rows per core
C = N // CGRID               # 4096 cols per core
NIT = R // 128               # 16 row tiles
CHUNK = 2048                 # col chunk processed per inner step
NCTP = C // CHUNK            # 4 col chunks
TEMP = 0.07
EPS = 1e-8
SHIFT = 1.0                  # fixed exp shift; cos sim <= 1
NEG = -1.0e9
GPSIMD_STT = False           # gpsimd lacks scalar_tensor_tensor codegen

F32 = mybir.dt.float32
F32R = mybir.dt.float32r
BF16 = mybir.dt.bfloat16
AX = mybir.AxisListType
OP = mybir.AluOpType
AF = mybir.ActivationFunctionType


def _label_encode(lab):
    """Map class ids 0..999 to distinct bf16-exact floats."""
    lab = np.asarray(lab).astype(np.int64)
    return ((128 + (lab % 128)) * (2.0 ** (lab // 128))).astype(np.float32)


def build_nc():
    nc = bacc.Bacc(
        "TRN2",
        target_bir_lowering=False,
        debug=False,
        num_devices=NCORES,
    )

    lhs = nc.declare_dram_parameter("lhs", [D, R], F32R, isOutput=False)
    rhsD = nc.declare_dram_parameter("rhsD", [D, C], F32R, isOutput=False)
    rhsL = nc.declare_dram_parameter("rhsL", [D, C], F32R, isOutput=False)
    labs = nc.declare_dram_parameter("labs", [128, NIT], F32, isOutput=False)
    labc = nc.declare_dram_parameter("labc", [128, C], BF16, isOutput=False)
    mA = nc.declare_dram_parameter("mA", [128, 1152], F32R, isOutput=False)
    mB = nc.declare_dram_parameter("mB", [128, 1152], F32R, isOutput=False)
    eyeK = nc.declare_dram_parameter("eyeK", [128, 128], F32R, isOutput=False)

    outs = {
        name: nc.declare_dram_parameter(name, [128, NIT], F32, isOutput=True)
        for name in ("nf", "df", "mf", "ne", "de", "me")
    }

    with tile.TileContext(nc) as tc:
        with (
            tc.tile_pool(name="const", bufs=1) as const,
            tc.tile_pool(name="rhsp", bufs=2) as rhsp,
            tc.tile_pool(name="psum", bufs=2, space="PSUM") as psum,
            tc.tile_pool(name="lp", bufs=2) as lp,
            tc.tile_pool(name="mlp", bufs=1) as mlp,
            tc.tile_pool(name="mtp", bufs=1) as mtp,
            tc.tile_pool(name="statp", bufs=1) as statp,
            tc.tile_pool(name="outp", bufs=1) as outp,
        ):
            dma = nc.default_dma_engine

            # lhs loaded in 512-col chunks so the first matmuls can start
            # after ~3 MiB of DMA instead of the full 12 MiB.
            lhs_sb = []
            for kc in range(4):
                t = const.tile([128, R], F32R, tag=f"lhs{kc}")
                dma.dma_start(
                    out=t[:, 0:512], in_=lhs[kc * 128:(kc + 1) * 128, 0:512]
                )
                lhs_sb.append(t)
            eye_sb = const.tile([128, 128], F32R, tag="eyeK")
            dma.dma_start(out=eye_sb, in_=eyeK[:, :])
            mA_sb = const.tile([128, 1152], F32R, tag="mA")
            dma.dma_start(out=mA_sb, in_=mA[:, :])
            mB_sb = const.tile([128, 1152], F32R, tag="mB")
            dma.dma_start(out=mB_sb, in_=mB[:, :])
            labs_sb = const.tile([128, NIT], F32, tag="labs")
            labc_sb = const.tile([128, C], BF16, tag="labc")
            bias_sb = const.tile([128, 1], F32, tag="expbias")
            nc.vector.memset(bias_sb, -SHIFT / TEMP)

            # per-(it, ctp) stat slots, reduced to per-it at the end
            stats = {}
            for sname in ("f", "e"):
                for kind in ("n", "d", "m"):
                    t = statp.tile([128, NIT * NCTP], F32, tag=f"st_{kind}{sname}")
                    stats[kind + sname] = t

            for ctp in range(NCTP):
                rhs_t = {}
                for mname, dram in (("f", rhsD), ("e", rhsL)):
                    tiles = []
                    for kc in range(4):
                        t = rhsp.tile([128, CHUNK], F32R, tag=f"rhs_{mname}{kc}")
                        dma.dma_start(
                            out=t,
                            in_=dram[kc * 128:(kc + 1) * 128,
                                     ctp * CHUNK:(ctp + 1) * CHUNK],
                        )
                        tiles.append(t)
                    rhs_t[mname] = tiles

                if ctp == 0:
                    # deferred loads: needed only a few tiles into the loop
                    dma.dma_start(out=labs_sb, in_=labs[:, :])
                    dma.dma_start(out=labc_sb, in_=labc[:, :])
                    for kc in range(4):
                        for part in range(1, 4):
                            sl = slice(part * 512, (part + 1) * 512)
                            dma.dma_start(
                                out=lhs_sb[kc][:, sl],
                                in_=lhs[kc * 128:(kc + 1) * 128, sl],
                            )

                for it in range(NIT):
                    slot = slice(it * NCTP + ctp, it * NCTP + ctp + 1)

                    for sname in ("f", "e"):
                        ps = psum.tile([128, CHUNK], F32, tag="ps")
                        nnt = CHUNK // 512
                        # kc-outer order: the nt-regions of one kc share the
                        # same stationary lhsT, so LDWEIGHTS amortizes.
                        for kc in range(3):
                            for nt in range(nnt):
                                reg = ps[:, nt * 512:(nt + 1) * 512]
                                nc.tensor.matmul(
                                    reg,
                                    lhsT=lhs_sb[kc][:, it * 128:(it + 1) * 128],
                                    rhs=rhs_t[sname][kc][:, nt * 512:(nt + 1) * 512],
                                    start=(kc == 0),
                                    stop=False,
                                )
                        for nt in range(nnt):
                            ct = ctp * nnt + nt  # global 512-col tile
                            # Diagonal mask membership for the f2f strip:
                            # variant A targets ct = it//4 (row shard even),
                            # variant B targets ct = 4 + it//4 (row shard odd).
                            mask_sb = None
                            if sname == "f":
                                if ct == it // 4:
                                    mask_sb = mA_sb
                                elif ct == 4 + it // 4:
                                    mask_sb = mB_sb
                            reg = ps[:, nt * 512:(nt + 1) * 512]
                            nc.tensor.matmul(
                                reg,
                                lhsT=lhs_sb[3][:, it * 128:(it + 1) * 128],
                                rhs=rhs_t[sname][3][:, nt * 512:(nt + 1) * 512],
                                start=False,
                                stop=(mask_sb is None),
                            )
                            if mask_sb is not None:
                                start_col = 512 - 128 * (it % 4)
                                nc.tensor.matmul(
                                    reg,
                                    lhsT=eye_sb,
                                    rhs=mask_sb[:, start_col:start_col + 512],
                                    start=False,
                                    stop=True,
                                )

                        l_t = lp.tile([128, CHUNK], BF16, tag="l")
                        nc.scalar.activation(
                            out=l_t,
                            in_=ps,
                            func=AF.Exp,
                            bias=bias_sb,
                            scale=1.0 / TEMP,
                            accum_out=stats["d" + sname][:, slot],
                        )
                        # row max of l' (bf16, 2x TT tree); host recovers
                        # Ms = SHIFT + T*ln(max_l)
                        m1 = mtp.tile([128, 1024], BF16, tag="m1")
                        nc.vector.tensor_tensor(
                            out=m1, in0=l_t[:, :1024], in1=l_t[:, 1024:],
                            op=OP.max,
                        )
                        m2 = mtp.tile([128, 512], BF16, tag="m2")
                        nc.vector.tensor_tensor(
                            out=m2, in0=m1[:, :512], in1=m1[:, 512:],
                            op=OP.max,
                        )
                        m3 = mtp.tile([128, 256], BF16, tag="m3")
                        nc.vector.tensor_tensor(
                            out=m3, in0=m2[:, :256], in1=m2[:, 256:],
                            op=OP.max,
                        )
                        nc.vector.tensor_reduce(
                            out=stats["m" + sname][:, slot],
                            in_=m3,
                            axis=AX.X,
                            op=OP.max,
                        )
                        # num partial: ((labc == labs_it) * l) summed, one op
                        eng = nc.gpsimd if (GPSIMD_STT and sname == "e") else nc.vector
                        ml_t = mlp.tile([128, CHUNK], BF16, tag="ml")
                        eng.scalar_tensor_tensor(
                            out=ml_t,
                            in0=labc_sb[:, ctp * CHUNK:(ctp + 1) * CHUNK],
                            scalar=labs_sb[:, it:it + 1],
                            in1=l_t,
                            op0=OP.is_equal,
                            op1=OP.mult,
                            accum_out=stats["n" + sname][:, slot],
                        )

            # reduce the per-ctp slots and ship out
            for sname in ("f", "e"):
                for kind, op in (("n", OP.add), ("d", OP.add), ("m", OP.max)):
                    o = outp.tile([128, NIT], F32, tag=f"o_{kind}{sname}")
                    nc.vector.tensor_reduce(
                        out=o,
                        in_=stats[kind + sname].rearrange(
                            "p (a b) -> p a b", b=NCTP
                        ),
                        axis=AX.X,
                        op=op,
                    )
                    dma.dma_start(out=outs[kind + sname][:, :], in_=o)

    nc.finalize()
    return nc


_NC_CACHE = None


def _get_nc():
    global _NC_CACHE
    if _NC_CACHE is None:
        _NC_CACHE = build_nc()
    return _NC_CACHE


def _prep_inputs(embed_data, embed_label, label):
    xd = np.asarray(embed_data, dtype=np.float32)
    xl = np.asarray(embed_label, dtype=np.float32)
    lab = np.asarray(label)

    def norm(x):
        n = np.sqrt(np.sum(x.astype(np.float64) ** 2, axis=1, keepdims=True))
        n = np.maximum(n, EPS)
        return (x / n).astype(np.float32)

    xdT = np.ascontiguousarray(norm(xd).T)  # [D, N]
    xlT = np.ascontiguousarray(norm(xl).T)
    labf = _label_encode(lab)               # [N] f32, bf16-exact values

    import ml_dtypes

    eyeK = np.eye(128, dtype=np.float32)

    in_maps = []
    for k in range(NCORES):
        r, c = k % RGRID, k // RGRID
        rows = slice(R * r, R * (r + 1))
        cols = slice(C * c, C * (c + 1))

        labs = labf[rows].reshape(NIT, 128).T.copy()      # [128, NIT]
        labc = np.broadcast_to(
            labf[cols].astype(ml_dtypes.bfloat16), (128, C)
        ).copy()

        mAb = np.zeros((128, 1152), dtype=np.float32)
        mBb = np.zeros((128, 1152), dtype=np.float32)
        if r // 2 == c:
            tgt = mAb if r % 2 == 0 else mBb
            tgt[np.arange(128), 512 + np.arange(128)] = NEG

        in_maps.append({
            "lhs": np.ascontiguousarray(xdT[:, rows]),
            "rhsD": np.ascontiguousarray(xdT[:, cols]),
            "rhsL": np.ascontiguousarray(xlT[:, cols]),
            "labs": np.ascontiguousarray(labs),
            "labc": labc,
            "mA": mAb,
            "mB": mBb,
            "eyeK": eyeK,
        })
    return in_maps


def _combine(results):
    """Host combine of per-core shard stats -> scalar loss (fp64)."""
    # stats[name][r][c] = [128, NIT]
    def get(name):
        out = np.empty((RGRID, CGRID, 128, NIT), dtype=np.float64)
        for k in range(NCORES):
            r, c = k % RGRID, k // RGRID
            out[r, c] = results[k][name].astype(np.float64)
        return out

    nf, df, mf = get("nf"), get("df"), get("mf")
    ne, de, me = get("ne"), get("de"), get("me")

    # row g = 2048 r + 128 it + p  <->  [r, c, p, it]
    # mf/me hold max_j l' = exp((Ms - SHIFT)/T); the reference weight
    # e^{(SHIFT - Ms)/T} is just its reciprocal.
    Mlf = np.max(mf, axis=1)           # [RGRID, 128, NIT]
    Mle = np.max(me, axis=1)
    Af = np.sum(nf, axis=1)
    Cf = np.sum(df, axis=1)
    Ae = np.sum(ne, axis=1)
    Ce = np.sum(de, axis=1)

    wf = 1.0 / Mlf
    we = 1.0 / Mle
    num = we * Ae + wf * Af
    den = we * Ce + wf * Cf
    row_loss = np.log(den) - np.log(num)
    return np.float32(np.mean(row_loss))


def kernel(embed_data, embed_label, label):
    nc = _get_nc()
    in_maps = _prep_inputs(embed_data, embed_label, label)
    res = run_bass_kernel_spmd(nc, in_maps, list(range(NCORES)))
    return _combine(res.results)


if __name__ == "__main__":
    rng = np.random.default_rng(0)
    ed = rng.standard_normal((N, D), dtype=np.float32)
    el = rng.standard_normal((N, D), dtype=np.float32)
    lb = rng.integers(0, 1000, N)
    print(kernel(ed, el, lb))
